# revision 10
# baseline (speedup 1.0000x reference)
"""Trainium2 Bass kernel for nn_Conv1x1Net (StyleGAN-style modulated 1x1 conv net).

Network (per reference):
  for l in 0..7:
    styles = ws[l] @ (affine_w[l].T * gain) + affine_b[l]            # [B,CIN]
    base   = lrelu(mod_conv1x1(base, conv_w[l], conv_b[l], styles))  # [B,H,W,C]
  sigma = |base @ sigma_w.T + sigma_b|        # [B,H,W]
  feat  = (base @ remap_w.T + remap_b).transpose(0,3,1,2)

Sharding: 8 cores; core c handles sample b=c//2, rows h0=(c%2)*64 — 8192
spatial positions each. Each sample has its own demodulated weight so this is
embarrassingly parallel; the per-sample weight prep is recomputed on every
core (it is tiny).

Device layout: activations live channels-on-partitions as xT[ci, pos] so each
layer is out[o,pos] = vT[ci,o].T @ xT[ci,pos] with the demod scale d[o] and
bias folded into one ScalarE Lrelu epilogue (out = lrelu(g[o]*psum + b[o])).
Math: with r[o]=rsqrt(mean_i w[o,i]^2), c=rsqrt(mean styles^2),
v[i,o]=wT[i,o]*styles[b,i] (no r, no c), q[o]=sum_i v^2, S[o]=sum_i wT^2:
  g[o] = r*c*d = 16*c / sqrt(256*c^2*q[o] + 1e-8*S[o])
Matmuls run in float32r (fp32 storage, PE rounds internally; ~1e-4 relative,
full bf16-rate on the PE array).
"""

import numpy as np

import concourse.bacc as bacc
import concourse.tile as tile
from concourse import mybir
from concourse.bass_utils import run_bass_kernel_spmd

F32 = mybir.dt.float32
F32R = mybir.dt.float32r
LRELU = mybir.ActivationFunctionType.Lrelu
IDENT = mybir.ActivationFunctionType.Identity
SQRT = mybir.ActivationFunctionType.Sqrt
ABS = mybir.ActivationFunctionType.Abs
MULT = mybir.AluOpType.mult
ADD = mybir.AluOpType.add

N_CORES = 8
D, B, H, W = 8, 4, 128, 128
CIN, COUT, WDIM = 256, 256, 128
NEG_SLOPE = 0.01
GAIN = 1.0 / np.sqrt(WDIM)
POS = (H // 2) * W            # positions per core = 8192
NCHUNK = 4                    # position chunks per core
CHUNK = POS // NCHUNK         # 2048
NSUB = CHUNK // 512           # 512-wide matmul slices per chunk
NT = 2                        # 128-channel tiles


def _build():
    """Build the per-core Bass program.

    The per-core sample's style column is always column 0: the host permutes
    the ws sample axis per core so each core sees its own sample first (the
    global styles RMS is permutation-invariant).
    """
    b_idx = 0
    nc = bacc.Bacc("TRN2", target_bir_lowering=False)

    xT_d = nc.dram_tensor("xT", [CIN, POS], F32R, kind="ExternalInput")
    wsT_d = nc.dram_tensor("wsT", [D, WDIM, B], F32R, kind="ExternalInput")
    afwT_d = nc.dram_tensor("afwT", [D, WDIM, CIN], F32R, kind="ExternalInput")
    afb_d = nc.dram_tensor("afb", [D, CIN], F32, kind="ExternalInput")
    cwT_d = nc.dram_tensor("cwT", [D, CIN, COUT], F32, kind="ExternalInput")
    cb_d = nc.dram_tensor("cb", [D, COUT], F32, kind="ExternalInput")
    rwT_d = nc.dram_tensor("rwT", [CIN, COUT], F32R, kind="ExternalInput")
    rb_d = nc.dram_tensor("rb", [COUT], F32, kind="ExternalInput")
    swT_d = nc.dram_tensor("swT", [CIN], F32R, kind="ExternalInput")
    ones_d = nc.dram_tensor("ones", [128, 1], F32R, kind="ExternalInput")
    sb_d = nc.dram_tensor("sb", [1, 1], F32, kind="ExternalInput")
    featT_d = nc.dram_tensor("featT", [COUT, POS], F32, kind="ExternalOutput")
    sigma_d = nc.dram_tensor("sigma", [1, POS], F32, kind="ExternalOutput")

    with tile.TileContext(nc) as tc:
        with (
            tc.tile_pool(name="consts", bufs=1) as consts,
            tc.tile_pool(name="weights", bufs=1) as wpool,
            tc.tile_pool(name="acts", bufs=1) as apool,
            tc.tile_pool(name="temps", bufs=1) as temps,
            tc.tile_pool(name="fout", bufs=2) as fpool,
            tc.tile_pool(name="sout", bufs=2) as spool,
            tc.tile_pool(name="dram", bufs=2, space="DRAM") as dpool,
            tc.tile_pool(name="psum", bufs=2, space="PSUM") as psum,
        ):
            # ---- constants / bias columns ----
            ones_r = consts.tile([128, 1], F32R, tag="ones_r")
            nc.sync.dma_start(out=ones_r, in_=ones_d[:, :])
            ones_f = consts.tile([128, 1], F32, tag="ones_f")
            nc.vector.memset(ones_f, 1.0)

            cb_cols = consts.tile([128, D, NT], F32, tag="cb_cols")
            nc.gpsimd.dma_start(
                out=cb_cols, in_=cb_d.rearrange("l (t p) -> p l t", p=128)
            )
            afb_cols = consts.tile([128, D, NT], F32, tag="afb_cols")
            nc.gpsimd.dma_start(
                out=afb_cols, in_=afb_d.rearrange("l (t p) -> p l t", p=128)
            )
            rb_cols = consts.tile([128, NT], F32, tag="rb_cols")
            nc.gpsimd.dma_start(out=rb_cols, in_=rb_d.rearrange("(t p) -> p t", p=128))
            sw_cols = consts.tile([128, NT], F32R, tag="sw_cols")
            nc.gpsimd.dma_start(out=sw_cols, in_=swT_d.rearrange("(t p) -> p t", p=128))
            sb_t = consts.tile([1, 1], F32, tag="sb_t")
            nc.sync.dma_start(out=sb_t, in_=sb_d[:, :])

            rw_sb = []
            for it in range(NT):
                t = wpool.tile([128, COUT], F32R, tag=f"rw{it}")
                nc.sync.dma_start(out=t, in_=rwT_d[it * 128 : (it + 1) * 128, :])
                rw_sb.append(t)

            # ---- phase A: per-layer weight prep (all small) ----
            vT = [[None] * NT for _ in range(D)]
            g_cols = [None] * D
            for l in range(D):
                cw_sb = []
                for it in range(NT):
                    t = temps.tile([128, COUT], F32, tag=f"cw{it}")
                    nc.sync.dma_start(
                        out=t, in_=cwT_d[l, it * 128 : (it + 1) * 128, :]
                    )
                    cw_sb.append(t)
                ws_sb = temps.tile([WDIM, B], F32R, tag="ws")
                nc.sync.dma_start(out=ws_sb, in_=wsT_d[l])
                afw_sb = temps.tile([WDIM, CIN], F32R, tag="afw")
                nc.sync.dma_start(out=afw_sb, in_=afwT_d[l])

                # stylesT[ci, b] = gain * (afwT.T @ wsT) + affine_b
                styles = []
                for ct in range(NT):
                    ps = psum.tile([128, B], F32, tag="mm")
                    nc.tensor.matmul(
                        ps, afw_sb[:, ct * 128 : (ct + 1) * 128], ws_sb,
                        start=True, stop=True,
                    )
                    st = temps.tile([128, B], F32, tag=f"st{ct}")
                    nc.scalar.activation(
                        out=st, in_=ps, func=IDENT,
                        bias=afb_cols[:, l, ct : ct + 1], scale=float(GAIN),
                    )
                    styles.append(st)

                # c = rsqrt(mean(styles^2)) over all [CIN, B]
                ps_c = psum.tile([1, 1], F32, tag="mm")
                for ct in range(NT):
                    rs = temps.tile([128, 1], F32, tag=f"rs{ct}")
                    sq = temps.tile([128, B], F32, tag=f"sq{ct}")
                    nc.vector.tensor_tensor(
                        out=sq, in0=styles[ct], in1=styles[ct], op=MULT,
                    )
                    nc.vector.tensor_reduce(
                        out=rs, in_=sq, axis=mybir.AxisListType.X, op=ADD
                    )
                    nc.tensor.matmul(
                        ps_c, ones_f, rs, start=(ct == 0), stop=(ct == 1)
                    )
                inv_c = temps.tile([1, 1], F32, tag="inv_c")
                nc.scalar.activation(
                    out=inv_c, in_=ps_c, func=SQRT, scale=1.0 / (B * CIN)
                )
                c_sb = temps.tile([1, 1], F32, tag="c_sb")
                nc.vector.reciprocal(out=c_sb, in_=inv_c)
                c2_sb = temps.tile([1, 1], F32, tag="c2_sb")
                nc.vector.tensor_tensor(out=c2_sb, in0=c_sb, in1=c_sb, op=MULT)

                # v[i,o] = cwT[i,o] * styles[i,b];  q[o]=sum_i v^2;  S[o]=sum_i cwT^2
                ps_q = psum.tile([1, COUT], F32, tag="mm")
                ps_s = psum.tile([1, COUT], F32, tag="mm")
                for it in range(NT):
                    v = wpool.tile([128, COUT], F32R, tag=f"v{l}_{it}")
                    nc.vector.tensor_scalar_mul(
                        v, cw_sb[it], styles[it][:, b_idx : b_idx + 1]
                    )
                    vT[l][it] = v
                    v2 = temps.tile([128, COUT], F32R, tag=f"v2_{it}")
                    nc.vector.tensor_tensor(
                        out=v2, in0=v[:].bitcast(F32), in1=v[:].bitcast(F32), op=MULT
                    )
                    cw2 = temps.tile([128, COUT], F32R, tag=f"cw2_{it}")
                    nc.vector.tensor_tensor(
                        out=cw2, in0=cw_sb[it], in1=cw_sb[it], op=MULT
                    )
                    nc.tensor.matmul(
                        ps_q, ones_r, v2, start=(it == 0), stop=(it == 1)
                    )
                    nc.tensor.matmul(
                        ps_s, ones_r, cw2, start=(it == 0), stop=(it == 1)
                    )

                # g[o] = 16*c / sqrt(256*c^2*q[o] + 1e-8*S[o])
                u = temps.tile([1, COUT], F32, tag="u")
                nc.vector.tensor_scalar(
                    u, ps_q, c2_sb[0:1, 0:1], float(CIN), op0=MULT, op1=MULT
                )
                t_row = temps.tile([1, COUT], F32, tag="t_row")
                nc.vector.scalar_tensor_tensor(
                    out=t_row, in0=ps_s, scalar=1e-8, in1=u, op0=MULT, op1=ADD
                )
                sq_t = temps.tile([1, COUT], F32, tag="sq_t")
                nc.scalar.activation(out=sq_t, in_=t_row, func=SQRT, scale=1.0 / CIN)
                rec = temps.tile([1, COUT], F32, tag="rec")
                nc.vector.reciprocal(out=rec, in_=sq_t)
                g_row = temps.tile([1, COUT], F32, tag="g_row")
                nc.vector.tensor_scalar_mul(g_row, rec, c_sb[0:1, 0:1])

                g_b = dpool.tile([1, COUT], F32, tag="g_b")
                nc.sync.dma_start(out=g_b, in_=g_row)
                gc = wpool.tile([128, NT], F32, tag=f"g{l}")
                nc.gpsimd.dma_start(
                    out=gc, in_=g_b[0, :].rearrange("(t p) -> p t", p=128)
                )
                g_cols[l] = gc

            # ---- phase B: load activations (channels-on-partitions) ----
            xa = [[None] * NCHUNK for _ in range(NT)]
            xb = [[None] * NCHUNK for _ in range(NT)]
            for it in range(NT):
                for ch in range(NCHUNK):
                    ta = apool.tile(
                        [128, CHUNK], F32R, tag=f"xa{it}_{ch}", name=f"xa{it}_{ch}"
                    )
                    nc.sync.dma_start(
                        out=ta,
                        in_=xT_d[
                            it * 128 : (it + 1) * 128,
                            ch * CHUNK : (ch + 1) * CHUNK,
                        ],
                    )
                    xa[it][ch] = ta
                    xb[it][ch] = apool.tile(
                        [128, CHUNK], F32R, tag=f"xb{it}_{ch}", name=f"xb{it}_{ch}"
                    )

            # ---- phase C: 8 modulated-conv layers ----
            cur, nxt = xa, xb
            for l in range(D):
                for ch in range(NCHUNK):
                    for ot in range(NT):
                        ps = psum.tile([128, CHUNK], F32, tag="mm")
                        for it in range(NT):
                            lhsT = vT[l][it][:, ot * 128 : (ot + 1) * 128]
                            for nt in range(NSUB):
                                nc.tensor.matmul(
                                    ps[:, nt * 512 : (nt + 1) * 512],
                                    lhsT,
                                    cur[it][ch][:, nt * 512 : (nt + 1) * 512],
                                    start=(it == 0),
                                    stop=(it == NT - 1),
                                )
                        nc.scalar.activation(
                            out=nxt[ot][ch], in_=ps, func=LRELU,
                            bias=cb_cols[:, l, ot : ot + 1],
                            scale=g_cols[l][:, ot : ot + 1],
                            alpha=NEG_SLOPE,
                        )
                cur, nxt = nxt, cur

            # ---- phase D: remap + sigma ----
            for ch in range(NCHUNK):
                for ot in range(NT):
                    ps = psum.tile([128, CHUNK], F32, tag="mm")
                    for it in range(NT):
                        lhsT = rw_sb[it][:, ot * 128 : (ot + 1) * 128]
                        for nt in range(NSUB):
                            nc.tensor.matmul(
                                ps[:, nt * 512 : (nt + 1) * 512],
                                lhsT,
                                cur[it][ch][:, nt * 512 : (nt + 1) * 512],
                                start=(it == 0),
                                stop=(it == NT - 1),
                            )
                    fo = fpool.tile([128, CHUNK], F32, tag="fo")
                    nc.scalar.activation(
                        out=fo, in_=ps, func=IDENT,
                        bias=rb_cols[:, ot : ot + 1], scale=1.0,
                    )
                    nc.sync.dma_start(
                        out=featT_d[
                            ot * 128 : (ot + 1) * 128,
                            ch * CHUNK : (ch + 1) * CHUNK,
                        ],
                        in_=fo,
                    )
                ps_sg = psum.tile([1, CHUNK], F32, tag="mm")
                for it in range(NT):
                    for nt in range(NSUB):
                        nc.tensor.matmul(
                            ps_sg[:, nt * 512 : (nt + 1) * 512],
                            sw_cols[:, it : it + 1],
                            cur[it][ch][:, nt * 512 : (nt + 1) * 512],
                            start=(it == 0),
                            stop=(it == NT - 1),
                        )
                so = spool.tile([1, CHUNK], F32, tag="so", bufs=1)
                nc.scalar.activation(
                    out=so, in_=ps_sg, func=ABS, bias=sb_t[0:1, 0:1], scale=1.0
                )
                nc.sync.dma_start(
                    out=sigma_d[0:1, ch * CHUNK : (ch + 1) * CHUNK], in_=so
                )

    nc.finalize()
    return nc


_NC_CACHE = {}
_last_in_maps = None


def kernel(x, ws, affine_w, affine_b, conv_w, conv_b, sigma_w, sigma_b,
           remap_w, remap_b):
    x = np.ascontiguousarray(x, np.float32)
    ws = np.asarray(ws, np.float32)
    afwT = np.ascontiguousarray(np.transpose(affine_w, (0, 2, 1)), np.float32)
    cwT = np.ascontiguousarray(np.transpose(conv_w, (0, 2, 1)), np.float32)
    rwT = np.ascontiguousarray(np.asarray(remap_w, np.float32).T)
    swT = np.ascontiguousarray(np.asarray(sigma_w, np.float32).reshape(COUT))
    sb = np.asarray(sigma_b, np.float32).reshape(1, 1)
    afb = np.ascontiguousarray(affine_b, np.float32)
    cb = np.ascontiguousarray(conv_b, np.float32)
    rb = np.ascontiguousarray(remap_b, np.float32)

    in_maps = []
    for core in range(N_CORES):
        b = core // 2
        h0 = (core % 2) * (H // 2)
        xs = np.ascontiguousarray(
            x[b, h0 : h0 + H // 2].reshape(POS, CIN).T
        )
        # put this core's sample first in the ws sample axis; the global
        # styles RMS is permutation-invariant so only column 0's identity
        # matters.
        perm = [b] + [j for j in range(B) if j != b]
        wsT = np.ascontiguousarray(np.transpose(ws[:, perm, :], (0, 2, 1)))
        in_maps.append(
            dict(xT=xs, wsT=wsT, afwT=afwT, afb=afb, cwT=cwT, cb=cb,
                 rwT=rwT, rb=rb, swT=swT, sb=sb,
                 ones=np.ones((128, 1), np.float32))
        )

    global _last_in_maps
    _last_in_maps = in_maps
    if "nc" not in _NC_CACHE:
        _NC_CACHE["nc"] = _build()
    res = run_bass_kernel_spmd(
        _NC_CACHE["nc"], in_maps, core_ids=list(range(N_CORES)), trace=False
    )
    results = res.results

    feat = np.empty((B, COUT, H, W), np.float32)
    sigma = np.empty((B, H, W), np.float32)
    for core in range(N_CORES):
        b = core // 2
        h0 = (core % 2) * (H // 2)
        feat[b, :, h0 : h0 + H // 2, :] = results[core]["featT"].reshape(
            COUT, H // 2, W
        )
        sigma[b, h0 : h0 + H // 2, :] = results[core]["sigma"].reshape(H // 2, W)
    return feat, sigma
